# revision 7
# baseline (speedup 1.0000x reference)
"""GCN layer (degree-normalized SpMM + dense matmul) on 8 Trainium2 cores.

out = D^-1/2 A D^-1/2 feat W + b, A built from 600K (src, dst) edges.

Sharding: destination nodes across 8 cores (12500 each). Within a core,
nodes are greedily re-packed into 98 windows of 128 so that each
(window, src-bank) pair holds <= 256 incoming edges (the measured max is
~196 vs mean 191). feat is replicated per core, split into 4 row-banks
of 25000 (int16 index range for the custom gather ucode).

Device pipeline per window:
  - 4x dma_gather (one per bank, one SWDGE queue each -> parallel Q7
    descriptor generation) pull the window's source feature rows into
    SBUF as 8 chunks of [128 edges, 128 feat].
  - Per chunk, the vector engine builds onehot[e, v] =
    (iota[v] == dst_slot[e]) * norm_src[e] in one tensor_scalar op.
  - TensorE accumulates agg^T[din, v] += X_chunk^T @ onehot in PSUM.
  - TensorE computes (agg^T)^T @ W -> [v, dout] in PSUM; the scalar
    engine applies the per-node norm[dst] scale on the PSUM->SBUF copy;
    the vector engine adds the (broadcast) bias; HWDGE writes the
    window's 128 output rows.

Host-side work is shard construction only: degree histograms + rsqrt
norms, node re-packing, edge bucketing/padding, constant tables, and
the inverse node permutation at unshard.
"""

import numpy as np

N_NODES = 100000
N_EDGES = 600000
D = 128
NC = 8            # cores
NPC = 12500       # nodes per core
P = 128           # partitions / window size
W = 98            # windows per core
NB = 4            # feat banks
BS = 25000        # bank size (int16-addressable)
CAP = 256         # max edges per (window, bank) -> 2 chunks of 128
CPW = NB * 2      # chunks per window
XG_BUFS = 4


def _build_bass(caps, rep=None):
    """caps: [W, NB] int array of static per-(window, bank) index counts
    (128 < caps <= 256, shared across cores). rep: wrap the window loop
    in a hardware For_i for benchmarking."""
    import concourse.bacc as bacc
    import concourse.bass as bass
    import concourse.mybir as mybir
    import concourse.tile as tile

    f32 = mybir.dt.float32
    i16 = mybir.dt.int16

    # idx column layout: per (w, b) a block of ceil(cap/16) int16 columns,
    # rounded to even so every block starts 4B-aligned
    ncols = [(-(-int(caps[w, b]) // 16) + 1) // 2 * 2 for w in range(W) for b in range(NB)]
    col0 = np.concatenate([[0], np.cumsum(ncols)])
    tot_cols = int(col0[-1])

    nc = bacc.Bacc(
        None,
        target_bir_lowering=False,
        dynamic_dma_scratch_size=32768,
        num_swdge_queues=4,
    )
    feat_b = [
        nc.declare_dram_parameter(f"feat{b}", [BS, D], f32, isOutput=False)
        for b in range(NB)
    ]
    w_d = nc.declare_dram_parameter("w", [D, D], f32, isOutput=False)
    biasb_d = nc.declare_dram_parameter("biasb", [P, D], f32, isOutput=False)
    iota_d = nc.declare_dram_parameter("iota", [P, P], f32, isOutput=False)
    idx_d = nc.declare_dram_parameter("idx", [P, tot_cols], i16, isOutput=False)
    dstc_d = nc.declare_dram_parameter("dstc", [P, W * CPW], f32, isOutput=False)
    sed_d = nc.declare_dram_parameter("sed", [P, W * CPW], f32, isOutput=False)
    normd_d = nc.declare_dram_parameter("normd", [P, W], f32, isOutput=False)
    out_d = nc.declare_dram_parameter("out", [W * P, D], f32, isOutput=True)

    with tile.TileContext(nc) as tc:
        with (
            tc.tile_pool(name="const", bufs=1) as cp,
            tc.tile_pool(name="xg", bufs=XG_BUFS) as xp,
            tc.tile_pool(name="oh", bufs=4) as ohp,
            tc.tile_pool(name="sb", bufs=4) as sbp,
            tc.tile_pool(name="osb", bufs=4) as obp,
            tc.tile_pool(name="ps1", bufs=4, space="PSUM") as pp1,
            tc.tile_pool(name="ps2", bufs=2, space="PSUM") as pp2,
        ):
            idx_sb = cp.tile([P, tot_cols], i16)
            nc.sync.dma_start(out=idx_sb[:], in_=idx_d[:])
            dstc_sb = cp.tile([P, W * CPW], f32)
            nc.sync.dma_start(out=dstc_sb[:], in_=dstc_d[:])
            sed_sb = cp.tile([P, W * CPW], f32)
            nc.sync.dma_start(out=sed_sb[:], in_=sed_d[:])
            normd_sb = cp.tile([P, W], f32)
            nc.sync.dma_start(out=normd_sb[:], in_=normd_d[:])
            iota_sb = cp.tile([P, P], f32)
            nc.sync.dma_start(out=iota_sb[:], in_=iota_d[:])
            biasb_sb = cp.tile([P, D], f32)
            nc.sync.dma_start(out=biasb_sb[:], in_=biasb_d[:])
            w_sb = cp.tile([D, D], f32)
            nc.sync.dma_start(out=w_sb[:], in_=w_d[:])

            import contextlib

            loop_cm = tc.For_i(0, rep, 1) if rep else contextlib.nullcontext()
            with loop_cm:
                for w_i in range(W):
                    xg = xp.tile([P, CPW * D], f32, tag="xg")
                    if rep is None and w_i < XG_BUFS:
                        # first use of each buf slot: clear so skipped
                        # trailing slots hold finite data (NaN safety)
                        nc.vector.memset(xg[:], 0.0)
                    for b in range(NB):
                        g = w_i * NB + b
                        n = int(caps[w_i, b])
                        nic = -(-n // 16)
                        nc.gpsimd.dma_gather(
                            out_ap=xg[:, b * 2 * D : (b + 1) * 2 * D].rearrange(
                                "p (c r) -> p c r", r=D
                            ),
                            in_ap=feat_b[b][:, :],
                            idxs_ap=idx_sb[:, int(col0[g]) : int(col0[g]) + nic],
                            num_idxs=n,
                            num_idxs_reg=n,
                            elem_size=D,
                            queue_num=b,
                        )
                    psA = pp1.tile([P, P], f32, tag="psA")
                    for j in range(CPW):
                        col = w_i * CPW + j
                        oh = ohp.tile([P, P], f32, tag="oh")
                        nc.vector.tensor_scalar(
                            out=oh[:],
                            in0=iota_sb[:],
                            scalar1=dstc_sb[:, col : col + 1],
                            scalar2=sed_sb[:, col : col + 1],
                            op0=mybir.AluOpType.is_equal,
                            op1=mybir.AluOpType.mult,
                        )
                        nc.tensor.matmul(
                            out=psA[:],
                            lhsT=xg[:, j * D : (j + 1) * D],
                            rhs=oh[:],
                            start=(j == 0),
                            stop=(j == CPW - 1),
                        )
                    aggT = sbp.tile([P, P], f32, tag="aggT")
                    nc.scalar.activation(
                        aggT[:], psA[:], mybir.ActivationFunctionType.Copy
                    )
                    psB = pp2.tile([P, D], f32, tag="psB")
                    nc.tensor.matmul(
                        out=psB[:], lhsT=aggT[:], rhs=w_sb[:], start=True, stop=True
                    )
                    hsb = obp.tile([P, D], f32, tag="hsb")
                    nc.scalar.activation(
                        hsb[:],
                        psB[:],
                        mybir.ActivationFunctionType.Copy,
                        scale=normd_sb[:, w_i : w_i + 1],
                    )
                    osb = obp.tile([P, D], f32, tag="osb")
                    nc.vector.tensor_add(out=osb[:], in0=hsb[:], in1=biasb_sb[:])
                    nc.sync.dma_start(
                        out=out_d[w_i * P : (w_i + 1) * P, :], in_=osb[:]
                    )
    nc.compile()
    return nc


def _prep_shards(feat, weight, bias, src, dst):
    feat = np.ascontiguousarray(np.asarray(feat, dtype=np.float32))
    weight = np.ascontiguousarray(np.asarray(weight, dtype=np.float32))
    bias = np.asarray(bias, dtype=np.float32)
    src = np.asarray(src, dtype=np.int64)
    dst = np.asarray(dst, dtype=np.int64)

    deg = np.bincount(dst, minlength=N_NODES)
    norm = (1.0 / np.sqrt(np.maximum(deg, 1.0))).astype(np.float32)
    bank = src // BS

    # per-node per-bank in-degree, for window packing
    d4 = np.zeros((N_NODES, NB), np.int64)
    for b in range(NB):
        np.add.at(d4[:, b], dst[bank == b], 1)

    # greedy re-pack of each core's nodes into W windows of <=128 nodes,
    # balancing the per-bank edge loads
    slot_of = np.full(N_NODES, -1, np.int32)   # node -> slot (0..127)
    win_of = np.full(N_NODES, -1, np.int32)    # node -> window (0..97)
    perm = np.full((NC, W * P), -1, np.int64)  # (core, w*128+p) -> node
    for m in range(NC):
        nodes = np.arange(m * NPC, (m + 1) * NPC)
        dv = d4[nodes]
        order = np.argsort(-dv.sum(1), kind="stable")
        loads = np.zeros((W, NB), np.int64)
        counts = np.zeros(W, np.int32)
        for i in order:
            cand = (loads + dv[i]).max(1)
            cand[counts >= P] = 1 << 40
            w = int(np.argmin(cand))
            n = nodes[i]
            win_of[n] = w
            slot_of[n] = counts[w]
            perm[m, w * P + counts[w]] = n
            loads[w] += dv[i]
            counts[w] += 1

    # bucket edges by (core, window, bank); position within bucket
    core_e = dst // NPC
    w_e = win_of[dst]
    key = (core_e * W + w_e) * NB + bank
    order = np.argsort(key, kind="stable")
    srcs, dsts, keys = src[order], dst[order], key[order]
    counts_e = np.bincount(keys, minlength=NC * W * NB)
    starts = np.zeros(NC * W * NB, np.int64)
    np.cumsum(counts_e[:-1], out=starts[1:])
    within = np.arange(N_EDGES, dtype=np.int64) - starts[keys]

    cnt3 = counts_e.reshape(NC, W, NB)
    caps = cnt3.max(axis=0)  # [W, NB] static counts shared by all cores
    assert caps.max() <= CAP, f"window/bank overflow: {caps.max()}"
    assert caps.min() > P, f"cap {caps.min()} <= 128 breaks uniform 2-chunk shape"

    # slot-dense arrays [NC, W, NB, CAP]
    idx_full = np.zeros((NC, W, NB, CAP), np.int16)
    dstc_full = np.full((NC, W, NB, CAP), 255.0, np.float32)
    sed_full = np.zeros((NC, W, NB, CAP), np.float32)
    flat = ((keys * CAP) + within).astype(np.int64)
    idx_full.reshape(-1)[flat] = (srcs % BS).astype(np.int16)
    dstc_full.reshape(-1)[flat] = slot_of[dsts]
    sed_full.reshape(-1)[flat] = norm[srcs]

    # gather idx layout: per (w,b) block of ceil(cap/16) cols rounded even,
    # value i at [i % 16, block + i // 16], replicated across 8 core-groups
    ncols = [(-(-int(caps[w, b]) // 16) + 1) // 2 * 2 for w in range(W) for b in range(NB)]
    col0 = np.concatenate([[0], np.cumsum(ncols)]).astype(np.int64)
    tot_cols = int(col0[-1])
    idx_dev = np.zeros((NC, 16, tot_cols), np.int16)
    for w in range(W):
        for b in range(NB):
            g = w * NB + b
            n = int(caps[w, b])
            blk = idx_full[:, w, b, : -(-n // 16) * 16].reshape(NC, -1, 16)
            idx_dev[:, :, col0[g] : col0[g] + blk.shape[1]] = blk.transpose(0, 2, 1)
    idx_dev = np.tile(idx_dev, (1, 8, 1))  # replicate to 128 partitions

    # onehot metadata [NC, 128, W*CPW]: chunk j of (w,b) -> column w*8+b*2+j
    dstc_dev = (
        dstc_full.reshape(NC, W, NB * 2, P).transpose(0, 3, 1, 2).reshape(NC, P, W * CPW)
    )
    sed_dev = (
        sed_full.reshape(NC, W, NB * 2, P).transpose(0, 3, 1, 2).reshape(NC, P, W * CPW)
    )

    norm_perm = np.where(perm >= 0, norm[np.maximum(perm, 0)], 0.0).astype(np.float32)
    normd = norm_perm.reshape(NC, W, P).transpose(0, 2, 1)  # [NC, 128, W]

    iota = np.broadcast_to(np.arange(P, dtype=np.float32), (P, P)).copy()
    biasb = np.broadcast_to(bias, (P, D)).copy()
    banks = [np.ascontiguousarray(feat[b * BS : (b + 1) * BS]) for b in range(NB)]

    in_maps = []
    for m in range(NC):
        im = {f"feat{b}": banks[b] for b in range(NB)}
        im.update(
            w=weight,
            biasb=biasb,
            iota=iota,
            idx=np.ascontiguousarray(idx_dev[m]),
            dstc=np.ascontiguousarray(dstc_dev[m]),
            sed=np.ascontiguousarray(sed_dev[m]),
            normd=np.ascontiguousarray(normd[m]),
        )
        in_maps.append(im)
    return in_maps, caps, perm


def kernel(feat, weight, bias, src, dst):
    from concourse.bass_utils import run_bass_kernel_spmd

    in_maps, caps, perm = _prep_shards(feat, weight, bias, src, dst)
    nc = _build_bass(caps)
    res = run_bass_kernel_spmd(nc, in_maps, list(range(NC)))
    out = np.empty((N_NODES, D), np.float32)
    for m in range(NC):
        o = res.results[m]["out"]
        mask = perm[m] >= 0
        out[perm[m][mask]] = o[mask]
    return out


# revision 13
# speedup vs baseline: 1.0114x; 1.0114x over previous
"""GCN layer (degree-normalized SpMM + dense matmul) on 8 Trainium2 cores.

out = D^-1/2 A D^-1/2 feat W + b, A built from 600K (src, dst) edges.

Sharding: destination nodes across 8 cores (12500 each). Within a core,
nodes are greedily re-packed into 98 windows of 128 so that each
(window, src-bank) pair holds <= 256 incoming edges (the measured max is
~196 vs mean 191). feat is replicated per core, split into 4 row-banks
of 25000 (int16 index range for the custom gather ucode).

Device pipeline per window:
  - 4x dma_gather (one per bank, one SWDGE queue each -> parallel Q7
    descriptor generation) pull the window's source feature rows into
    SBUF as 8 chunks of [128 edges, 128 feat].
  - Per chunk, the vector engine builds onehot[e, v] =
    (iota[v] == dst_slot[e]) * norm_src[e] in one tensor_scalar op.
  - TensorE accumulates agg^T[din, v] += X_chunk^T @ onehot in PSUM.
  - TensorE computes (agg^T)^T @ W -> [v, dout] in PSUM; the scalar
    engine applies the per-node norm[dst] scale on the PSUM->SBUF copy;
    the vector engine adds the (broadcast) bias; HWDGE writes the
    window's 128 output rows.

Host-side work is shard construction only: degree histograms + rsqrt
norms, node re-packing, edge bucketing/padding, constant tables, and
the inverse node permutation at unshard.
"""

import numpy as np

N_NODES = 100000
N_EDGES = 600000
D = 128
NC = 8            # cores
NPC = 12500       # nodes per core
P = 128           # partitions / window size
W = 98            # windows per core
NB = 4            # feat banks
BS = 25000        # bank size (int16-addressable)
CAP = 256         # max edges per (window, bank) -> 2 chunks of 128
CPW = NB * 2      # chunks per window
XG_BUFS = 16


def _build_bass(caps, rep=None, parts="all", bufs=None):
    """caps: [W, NB] int array of static per-(window, bank) index counts
    (128 < caps <= 256, shared across cores). rep: wrap the window loop
    in a hardware For_i for benchmarking. parts: 'all' | 'gather' |
    'compute' | 'onehot' | 'matmul' to isolate stages when benchmarking."""
    import concourse.bacc as bacc
    import concourse.bass as bass
    import concourse.mybir as mybir
    import concourse.tile as tile

    f32 = mybir.dt.float32
    i16 = mybir.dt.int16

    # idx column layout: per (w, b) a block of ceil(cap/16) int16 columns,
    # rounded to even so every block starts 4B-aligned
    ncols = [(-(-int(caps[w, b]) // 16) + 1) // 2 * 2 for w in range(W) for b in range(NB)]
    col0 = np.concatenate([[0], np.cumsum(ncols)])
    tot_cols = int(col0[-1])

    do_gather = parts in ("all", "gather")
    do_onehot = parts in ("all", "compute", "onehot")
    do_matmul = parts in ("all", "compute", "matmul")
    do_tail = parts in ("all", "compute")
    XB = bufs or XG_BUFS

    nc = bacc.Bacc(
        None,
        target_bir_lowering=False,
        dynamic_dma_scratch_size=32768,
        num_swdge_queues=4,
    )
    feat_b = [
        nc.declare_dram_parameter(f"feat{b}", [BS, D], f32, isOutput=False)
        for b in range(NB)
    ]
    w_d = nc.declare_dram_parameter("w", [D, D], f32, isOutput=False)
    biasb_d = nc.declare_dram_parameter("biasb", [P, D], f32, isOutput=False)
    iota_d = nc.declare_dram_parameter("iota", [P, P], f32, isOutput=False)
    idx_d = nc.declare_dram_parameter("idx", [P, tot_cols], i16, isOutput=False)
    dstc_d = nc.declare_dram_parameter("dstc", [P, W * CPW], f32, isOutput=False)
    sed_d = nc.declare_dram_parameter("sed", [P, W * CPW], f32, isOutput=False)
    normd_d = nc.declare_dram_parameter("normd", [P, W], f32, isOutput=False)
    out_d = nc.declare_dram_parameter("out", [W * P, D], f32, isOutput=True)

    with tile.TileContext(nc) as tc:
        with (
            tc.tile_pool(name="const", bufs=1) as cp,
            tc.tile_pool(name="xg", bufs=XB) as xp,
            tc.tile_pool(name="oh", bufs=8) as ohp,
            tc.tile_pool(name="sb", bufs=8) as sbp,
            tc.tile_pool(name="osb", bufs=8) as obp,
            tc.tile_pool(name="ps1", bufs=6, space="PSUM") as pp1,
            tc.tile_pool(name="ps2", bufs=2, space="PSUM") as pp2,
        ):
            idx_sb = cp.tile([P, tot_cols], i16)
            nc.sync.dma_start(out=idx_sb[:], in_=idx_d[:])
            dstc_sb = cp.tile([P, W * CPW], f32)
            nc.sync.dma_start(out=dstc_sb[:], in_=dstc_d[:])
            sed_sb = cp.tile([P, W * CPW], f32)
            nc.sync.dma_start(out=sed_sb[:], in_=sed_d[:])
            normd_sb = cp.tile([P, W], f32)
            nc.sync.dma_start(out=normd_sb[:], in_=normd_d[:])
            iota_sb = cp.tile([P, P], f32)
            nc.sync.dma_start(out=iota_sb[:], in_=iota_d[:])
            biasb_sb = cp.tile([P, D], f32)
            nc.sync.dma_start(out=biasb_sb[:], in_=biasb_d[:])
            w_sb = cp.tile([D, D], f32)
            nc.sync.dma_start(out=w_sb[:], in_=w_d[:])

            import contextlib

            loop_cm = tc.For_i(0, rep, 1) if rep else contextlib.nullcontext()
            with loop_cm:
                for w_i in range(W):
                    xg = xp.tile([P, CPW * D], f32, tag="xg")
                    if rep is None and w_i < XB:
                        # first use of each buf slot: clear so skipped
                        # trailing slots hold finite data (NaN safety)
                        nc.vector.memset(xg[:], 0.0)
                    if not do_gather and do_matmul:
                        # benchmark mode: xg needs a writer (Pool engine,
                        # off the critical DVE/PE path)
                        nc.gpsimd.memset(xg[:], 0.0)
                    for b in (range(NB) if do_gather else []):
                        g = w_i * NB + b
                        n = int(caps[w_i, b])
                        nic = -(-n // 16)
                        nc.gpsimd.dma_gather(
                            out_ap=xg[:, b * 2 * D : (b + 1) * 2 * D].rearrange(
                                "p (c r) -> p c r", r=D
                            ),
                            in_ap=feat_b[b][:, :],
                            idxs_ap=idx_sb[:, int(col0[g]) : int(col0[g]) + nic],
                            num_idxs=n,
                            num_idxs_reg=n,
                            elem_size=D,
                            queue_num=b,
                        )
                    psA = pp1.tile([P, P], f32, tag="psA")
                    for j in (range(CPW) if (do_onehot or do_matmul) else []):
                        col = w_i * CPW + j
                        oh = ohp.tile([P, P], f32, tag="oh")
                        if do_onehot:
                            nc.vector.tensor_scalar(
                            out=oh[:],
                            in0=iota_sb[:],
                            scalar1=dstc_sb[:, col : col + 1],
                            scalar2=sed_sb[:, col : col + 1],
                                op0=mybir.AluOpType.is_equal,
                                op1=mybir.AluOpType.mult,
                            )
                        if do_matmul:
                            nc.tensor.matmul(
                                out=psA[:],
                                lhsT=xg[:, j * D : (j + 1) * D],
                                rhs=oh[:],
                                start=(j == 0),
                                stop=(j == CPW - 1),
                            )
                    if not do_tail:
                        continue
                    aggT = sbp.tile([P, P], f32, tag="aggT")
                    nc.scalar.activation(
                        aggT[:], psA[:], mybir.ActivationFunctionType.Copy
                    )
                    psB = pp2.tile([P, D], f32, tag="psB")
                    nc.tensor.matmul(
                        out=psB[:], lhsT=aggT[:], rhs=w_sb[:], start=True, stop=True
                    )
                    hsb = obp.tile([P, D], f32, tag="hsb")
                    nc.scalar.activation(
                        hsb[:],
                        psB[:],
                        mybir.ActivationFunctionType.Copy,
                        scale=normd_sb[:, w_i : w_i + 1],
                    )
                    osb = obp.tile([P, D], f32, tag="osb")
                    nc.vector.tensor_add(out=osb[:], in0=hsb[:], in1=biasb_sb[:])
                    nc.sync.dma_start(
                        out=out_d[w_i * P : (w_i + 1) * P, :], in_=osb[:]
                    )
    nc.compile()
    return nc


def _prep_shards(feat, weight, bias, src, dst):
    feat = np.ascontiguousarray(np.asarray(feat, dtype=np.float32))
    weight = np.ascontiguousarray(np.asarray(weight, dtype=np.float32))
    bias = np.asarray(bias, dtype=np.float32)
    src = np.asarray(src, dtype=np.int64)
    dst = np.asarray(dst, dtype=np.int64)

    deg = np.bincount(dst, minlength=N_NODES)
    norm = (1.0 / np.sqrt(np.maximum(deg, 1.0))).astype(np.float32)
    bank = src // BS

    # per-node per-bank in-degree, for window packing
    d4 = np.zeros((N_NODES, NB), np.int64)
    for b in range(NB):
        np.add.at(d4[:, b], dst[bank == b], 1)

    # greedy re-pack of each core's nodes into W windows of <=128 nodes,
    # balancing the per-bank edge loads
    slot_of = np.full(N_NODES, -1, np.int32)   # node -> slot (0..127)
    win_of = np.full(N_NODES, -1, np.int32)    # node -> window (0..97)
    perm = np.full((NC, W * P), -1, np.int64)  # (core, w*128+p) -> node
    for m in range(NC):
        nodes = np.arange(m * NPC, (m + 1) * NPC)
        dv = d4[nodes]
        order = np.argsort(-dv.sum(1), kind="stable")
        loads = np.zeros((W, NB), np.int64)
        counts = np.zeros(W, np.int32)
        for i in order:
            cand = (loads + dv[i]).max(1)
            cand[counts >= P] = 1 << 40
            w = int(np.argmin(cand))
            n = nodes[i]
            win_of[n] = w
            slot_of[n] = counts[w]
            perm[m, w * P + counts[w]] = n
            loads[w] += dv[i]
            counts[w] += 1

    # bucket edges by (core, window, bank); position within bucket
    core_e = dst // NPC
    w_e = win_of[dst]
    key = (core_e * W + w_e) * NB + bank
    order = np.argsort(key, kind="stable")
    srcs, dsts, keys = src[order], dst[order], key[order]
    counts_e = np.bincount(keys, minlength=NC * W * NB)
    starts = np.zeros(NC * W * NB, np.int64)
    np.cumsum(counts_e[:-1], out=starts[1:])
    within = np.arange(N_EDGES, dtype=np.int64) - starts[keys]

    cnt3 = counts_e.reshape(NC, W, NB)
    caps = cnt3.max(axis=0)  # [W, NB] static counts shared by all cores
    assert caps.max() <= CAP, f"window/bank overflow: {caps.max()}"
    assert caps.min() > P, f"cap {caps.min()} <= 128 breaks uniform 2-chunk shape"

    # slot-dense arrays [NC, W, NB, CAP]
    idx_full = np.zeros((NC, W, NB, CAP), np.int16)
    dstc_full = np.full((NC, W, NB, CAP), 255.0, np.float32)
    sed_full = np.zeros((NC, W, NB, CAP), np.float32)
    flat = ((keys * CAP) + within).astype(np.int64)
    idx_full.reshape(-1)[flat] = (srcs % BS).astype(np.int16)
    dstc_full.reshape(-1)[flat] = slot_of[dsts]
    sed_full.reshape(-1)[flat] = norm[srcs]

    # gather idx layout: per (w,b) block of ceil(cap/16) cols rounded even,
    # value i at [i % 16, block + i // 16], replicated across 8 core-groups
    ncols = [(-(-int(caps[w, b]) // 16) + 1) // 2 * 2 for w in range(W) for b in range(NB)]
    col0 = np.concatenate([[0], np.cumsum(ncols)]).astype(np.int64)
    tot_cols = int(col0[-1])
    idx_dev = np.zeros((NC, 16, tot_cols), np.int16)
    for w in range(W):
        for b in range(NB):
            g = w * NB + b
            n = int(caps[w, b])
            blk = idx_full[:, w, b, : -(-n // 16) * 16].reshape(NC, -1, 16)
            idx_dev[:, :, col0[g] : col0[g] + blk.shape[1]] = blk.transpose(0, 2, 1)
    idx_dev = np.tile(idx_dev, (1, 8, 1))  # replicate to 128 partitions

    # onehot metadata [NC, 128, W*CPW]: chunk j of (w,b) -> column w*8+b*2+j
    dstc_dev = (
        dstc_full.reshape(NC, W, NB * 2, P).transpose(0, 3, 1, 2).reshape(NC, P, W * CPW)
    )
    sed_dev = (
        sed_full.reshape(NC, W, NB * 2, P).transpose(0, 3, 1, 2).reshape(NC, P, W * CPW)
    )

    norm_perm = np.where(perm >= 0, norm[np.maximum(perm, 0)], 0.0).astype(np.float32)
    normd = norm_perm.reshape(NC, W, P).transpose(0, 2, 1)  # [NC, 128, W]

    iota = np.broadcast_to(np.arange(P, dtype=np.float32), (P, P)).copy()
    biasb = np.broadcast_to(bias, (P, D)).copy()
    banks = [np.ascontiguousarray(feat[b * BS : (b + 1) * BS]) for b in range(NB)]

    in_maps = []
    for m in range(NC):
        im = {f"feat{b}": banks[b] for b in range(NB)}
        im.update(
            w=weight,
            biasb=biasb,
            iota=iota,
            idx=np.ascontiguousarray(idx_dev[m]),
            dstc=np.ascontiguousarray(dstc_dev[m]),
            sed=np.ascontiguousarray(sed_dev[m]),
            normd=np.ascontiguousarray(normd[m]),
        )
        in_maps.append(im)
    return in_maps, caps, perm


def kernel(feat, weight, bias, src, dst):
    from concourse.bass_utils import run_bass_kernel_spmd

    in_maps, caps, perm = _prep_shards(feat, weight, bias, src, dst)
    nc = _build_bass(caps)
    res = run_bass_kernel_spmd(nc, in_maps, list(range(NC)))
    out = np.empty((N_NODES, D), np.float32)
    for m in range(NC):
        o = res.results[m]["out"]
        mask = perm[m] >= 0
        out[perm[m][mask]] = o[mask]
    return out
